# revision 6
# baseline (speedup 1.0000x reference)
"""MoE routed dense layer (nn_MultiHeadDense): y[b] = x[b] @ W[idx[b]] + bias[idx[b]].

Full shapes: inputs [4096,1024] f32, indices [4096] int, kernel [8,1024,1024] f32,
bias [8,1024] f32 -> out [4096,1024] f32.

Sharding strategy (expert-parallel, H == n_cores == 8): core h owns expert h's
weight [1024,1024] and processes exactly the rows routed to expert h. The host
computes the per-expert row lists from `indices`, gathers each expert's rows
into a zero-padded transposed activation block XT_h [D, C] (C = padded max
group size), and scatters the per-core outputs back into the full [B, F]
result. This does 1/8th the FLOPs of the dense all-heads reference and loads
each expert weight exactly once, on exactly one core.

On-device per core: Y[c, f] = sum_k XT[k*128:(k+1)*128, c].T @ W[k*128:.., f]
accumulated in PSUM over the 8 k-tiles, bias added during the PSUM->SBUF
eviction. Matmuls run as float32r (full PE rate at N=512) on f32 data.
"""

from contextlib import ExitStack

import numpy as np

import concourse.bass as bass
import concourse.tile as tile
from concourse import bacc, mybir
from concourse.bass_utils import run_bass_kernel_spmd

F32 = mybir.dt.float32
F32R = mybir.dt.float32r

P = 128          # SBUF partitions / matmul tile edge
NTILE = 512      # matmul moving free dim (one fp32 PSUM bank)


def _build(nc: bass.Bass, C: int, D: int, F: int):
    xt = nc.dram_tensor("xt", (D, C), F32R, kind="ExternalInput").ap()
    w = nc.dram_tensor("w", (D, F), F32R, kind="ExternalInput").ap()
    b = nc.dram_tensor("b", (P, F), F32, kind="ExternalInput").ap()
    y = nc.dram_tensor("y", (C, F), F32, kind="ExternalOutput").ap()

    KT = D // P       # k-tiles (contraction)
    MT = C // P       # m-tiles (rows of this expert's batch)
    NT = F // NTILE   # n-tiles (output features)

    # DRAM views with the k-tile dim split out: [KT, P, cols]
    w3 = w.rearrange("(k p) f -> k p f", p=P)
    x3 = xt.rearrange("(k p) c -> k p c", p=P)

    with tile.TileContext(nc) as tc, ExitStack() as ctx:
        wp = ctx.enter_context(tc.tile_pool(name="wp", bufs=1))
        xp = ctx.enter_context(tc.tile_pool(name="xp", bufs=1))
        bp = ctx.enter_context(tc.tile_pool(name="bp", bufs=1))
        pp = ctx.enter_context(tc.tile_pool(name="pp", bufs=2 * NT, space="PSUM"))
        yp = ctx.enter_context(tc.tile_pool(name="yp", bufs=3))

        bias_t = bp.tile([P, F], F32)
        nc.scalar.dma_start(bias_t[:], b[:])

        # All of W resident as one [128, KT*F] tile; k-tile k lives at
        # columns [k*F, (k+1)*F). Loaded in KG-k-tile batches (~2 MB per
        # dma_start on the SP HWDGE ring) so transfers stay in the
        # high-efficiency regime while compute can start after the first.
        KG = 4
        w_t = wp.tile([P, KT * F], F32R)
        x_t = xp.tile([P, KT * C], F32R)
        for k0 in range(0, KT, KG):
            kn = min(KG, KT - k0)
            nc.sync.dma_start(
                w_t[:, k0 * F:(k0 + kn) * F],
                w3[k0:k0 + kn, :, :].rearrange("k p f -> p k f"),
            )
            nc.scalar.dma_start(
                x_t[:, k0 * C:(k0 + kn) * C],
                x3[k0:k0 + kn, :, :].rearrange("k p c -> p k c"),
            )

        for m in range(MT):
            ps = []
            for n in range(NT):
                pst = pp.tile([P, NTILE], F32, name=f"ps{m}_{n}", tag="ps")
                ps.append(pst)
            for k in range(KT):
                lhs = x_t[:, k * C + m * P:k * C + (m + 1) * P]
                for n in range(NT):
                    nc.tensor.matmul(
                        ps[n][:],
                        lhsT=lhs,
                        rhs=w_t[:, k * F + n * NTILE:k * F + (n + 1) * NTILE],
                        start=(k == 0),
                        stop=(k == KT - 1),
                    )
            yt = yp.tile([P, F], F32, name=f"y{m}", tag="y")
            for n in range(NT):
                nc.vector.tensor_add(
                    yt[:, n * NTILE:(n + 1) * NTILE],
                    ps[n][:],
                    bias_t[:, n * NTILE:(n + 1) * NTILE],
                )
            nc.scalar.dma_start(y[m * P:(m + 1) * P, :], yt[:])


LAST_PROFILE = {}


def kernel(inputs, indices, kernel, bias, _trace=False):
    x = np.ascontiguousarray(np.asarray(inputs), dtype=np.float32)
    idx = np.asarray(indices).astype(np.int64)
    wk = np.asarray(kernel, dtype=np.float32)
    bv = np.asarray(bias, dtype=np.float32)

    B, D = x.shape
    H, _, F = wk.shape

    rows = [np.nonzero(idx == h)[0] for h in range(H)]
    maxc = max(len(r) for r in rows)
    C = max(((maxc + P - 1) // P) * P, P)

    in_maps = []
    for h in range(H):
        r = rows[h]
        xt = np.zeros((D, C), dtype=np.float32)
        xt[:, :len(r)] = x[r].T
        in_maps.append({
            "xt": xt,
            "w": np.ascontiguousarray(wk[h]),
            "b": np.broadcast_to(bv[h], (P, F)).copy(),
        })

    nc = bacc.Bacc(
        "TRN2", target_bir_lowering=False, debug=False, num_devices=H
    )
    _build(nc, C, D, F)
    nc.compile()

    trace_kwargs = (
        {"trace": True, "trace_cores": list(range(H)), "stitch_traces": False}
        if _trace
        else {}
    )
    res = run_bass_kernel_spmd(nc, in_maps, core_ids=list(range(H)), **trace_kwargs)
    if _trace:
        LAST_PROFILE.clear()
        LAST_PROFILE.update(
            exec_time_ns=res.exec_time_ns,
            mean_exec_time_ns=res.mean_exec_time_ns,
            max_exec_time_core_id=res.max_exec_time_core_id,
            trace=res.instructions_and_trace[1] if res.instructions_and_trace else None,
            profile_json=res.profile_json,
        )

    out = np.empty((B, F), dtype=np.float32)
    for h in range(H):
        r = rows[h]
        out[r] = res.results[h]["y"][:len(r)]
    return out


# revision 9
# speedup vs baseline: 1.2520x; 1.2520x over previous
"""MoE routed dense layer (nn_MultiHeadDense): y[b] = x[b] @ W[idx[b]] + bias[idx[b]].

Full shapes: inputs [4096,1024] f32, indices [4096] int, kernel [8,1024,1024] f32,
bias [8,1024] f32 -> out [4096,1024] f32.

Sharding strategy (expert-parallel, H == n_cores == 8): core h owns expert h's
weight [1024,1024] and processes exactly the rows routed to expert h. The host
computes the per-expert row lists from `indices`, gathers each expert's rows
into a zero-padded transposed activation block XT_h [D, C] (C = padded max
group size), and scatters the per-core outputs back into the full [B, F]
result. This does 1/8th the FLOPs of the dense all-heads reference and loads
each expert weight exactly once, on exactly one core.

On-device per core: Y[c, f] = sum_k XT[k*128:(k+1)*128, c].T @ W[k*128:.., f]
accumulated in PSUM over the 8 k-tiles, bias added during the PSUM->SBUF
eviction. X and W are pre-cast to fp16 on the host (11-bit mantissa keeps the
absmax error ~1e-3 of output scale while halving HBM traffic and enabling the
fast PE weight-load path); accumulation stays fp32 in PSUM and bias is added
in fp32. The k loop is outermost (phased over output column halves) so the
matmul pipeline starts as soon as the first k-chunk of W/X lands.
"""

from contextlib import ExitStack

import numpy as np

import concourse.bass as bass
import concourse.tile as tile
from concourse import bacc, mybir
from concourse.bass_utils import run_bass_kernel_spmd

F32 = mybir.dt.float32
F16 = mybir.dt.float16

P = 128          # SBUF partitions / matmul tile edge
NTILE = 512      # matmul moving free dim (one fp32 PSUM bank)
KG = 2           # k-tiles per DMA chunk (pipeline granularity)


def _build(nc: bass.Bass, C: int, D: int, F: int):
    xt = nc.dram_tensor("xt", (D, C), F16, kind="ExternalInput").ap()
    w = nc.dram_tensor("w", (D, F), F16, kind="ExternalInput").ap()
    b = nc.dram_tensor("b", (P, F), F32, kind="ExternalInput").ap()
    y = nc.dram_tensor("y", (C, F), F32, kind="ExternalOutput").ap()

    KT = D // P       # k-tiles (contraction)
    MT = C // P       # m-tiles (rows of this expert's batch)
    NT = F // NTILE   # n-tiles (output features)
    KC = KT // KG     # k chunks

    # DRAM views with the k-tile dim split out: [KT, P, cols]
    w3 = w.rearrange("(k p) f -> k p f", p=P)
    x3 = xt.rearrange("(k p) c -> k p c", p=P)

    with tile.TileContext(nc) as tc, ExitStack() as ctx:
        wp = ctx.enter_context(tc.tile_pool(name="wp", bufs=KC))
        xp = ctx.enter_context(tc.tile_pool(name="xp", bufs=KC))
        bp = ctx.enter_context(tc.tile_pool(name="bp", bufs=1))
        pp = ctx.enter_context(tc.tile_pool(name="pp", bufs=MT, space="PSUM"))
        yp = ctx.enter_context(tc.tile_pool(name="yp", bufs=2 * NT))

        bias_t = bp.tile([P, F], F32)
        nc.scalar.dma_start(bias_t[:], b[:])

        # W and XT arrive in KG-k-tile chunks, interleaved on the SP HWDGE
        # ring so chunk c fully lands before chunk c+1 (ring is FIFO) and the
        # PE can start on chunk 0 while the rest stream in. Each chunk is a
        # separate tile so Tile's dependency tracking is per-chunk.
        w_c = []
        x_c = []
        for c in range(KC):
            wt = wp.tile([P, KG * F], F16, name=f"w{c}", tag="w")
            nc.sync.dma_start(
                wt[:], w3[c * KG:(c + 1) * KG, :, :].rearrange("k p f -> p k f")
            )
            w_c.append(wt)
            xtt = xp.tile([P, KG * C], F16, name=f"x{c}", tag="x")
            nc.sync.dma_start(
                xtt[:], x3[c * KG:(c + 1) * KG, :, :].rearrange("k p c -> p k c")
            )
            x_c.append(xtt)

        # Phase over output column halves; within a phase, k is outer and the
        # MT per-m PSUM banks accumulate in parallel, so only the first chunk
        # gates the pipeline start and W/X tiles are reused across phases.
        for n in range(NT):
            ps = []
            for m in range(MT):
                pst = pp.tile([P, NTILE], F32, name=f"ps{n}_{m}", tag="ps")
                ps.append(pst)
            for k in range(KT):
                c, ki = divmod(k, KG)
                rhs = w_c[c][:, ki * F + n * NTILE:ki * F + (n + 1) * NTILE]
                for m in range(MT):
                    nc.tensor.matmul(
                        ps[m][:],
                        lhsT=x_c[c][:, ki * C + m * P:ki * C + (m + 1) * P],
                        rhs=rhs,
                        start=(k == 0),
                        stop=(k == KT - 1),
                    )
            for m in range(MT):
                yt = yp.tile([P, NTILE], F32, name=f"yt{n}_{m}", tag="y")
                nc.vector.tensor_add(
                    yt[:], ps[m][:], bias_t[:, n * NTILE:(n + 1) * NTILE]
                )
                nc.scalar.dma_start(
                    y[m * P:(m + 1) * P, n * NTILE:(n + 1) * NTILE], yt[:]
                )


LAST_PROFILE = {}


def kernel(inputs, indices, kernel, bias, _trace=False):
    x = np.ascontiguousarray(np.asarray(inputs), dtype=np.float32)
    idx = np.asarray(indices).astype(np.int64)
    wk = np.asarray(kernel, dtype=np.float32)
    bv = np.asarray(bias, dtype=np.float32)

    B, D = x.shape
    H, _, F = wk.shape

    rows = [np.nonzero(idx == h)[0] for h in range(H)]
    maxc = max(len(r) for r in rows)
    C = max(((maxc + P - 1) // P) * P, P)

    in_maps = []
    for h in range(H):
        r = rows[h]
        xt = np.zeros((D, C), dtype=np.float16)
        xt[:, :len(r)] = x[r].T
        in_maps.append({
            "xt": xt,
            "w": wk[h].astype(np.float16),
            "b": np.broadcast_to(bv[h], (P, F)).copy(),
        })

    nc = bacc.Bacc(
        "TRN2", target_bir_lowering=False, debug=False, num_devices=H
    )
    _build(nc, C, D, F)
    nc.compile()

    trace_kwargs = (
        {"trace": True, "trace_cores": list(range(H)), "stitch_traces": False}
        if _trace
        else {}
    )
    res = run_bass_kernel_spmd(nc, in_maps, core_ids=list(range(H)), **trace_kwargs)
    if _trace:
        LAST_PROFILE.clear()
        LAST_PROFILE.update(
            exec_time_ns=res.exec_time_ns,
            mean_exec_time_ns=res.mean_exec_time_ns,
            max_exec_time_core_id=res.max_exec_time_core_id,
            trace=res.instructions_and_trace[1] if res.instructions_and_trace else None,
            profile_json=res.profile_json,
        )

    out = np.empty((B, F), dtype=np.float32)
    for h in range(H):
        r = rows[h]
        out[r] = res.results[h]["y"][:len(r)]
    return out


# revision 11
# speedup vs baseline: 1.2793x; 1.0218x over previous
"""MoE routed dense layer (nn_MultiHeadDense): y[b] = x[b] @ W[idx[b]] + bias[idx[b]].

Full shapes: inputs [4096,1024] f32, indices [4096] int, kernel [8,1024,1024] f32,
bias [8,1024] f32 -> out [4096,1024] f32.

Sharding strategy (expert-parallel, H == n_cores == 8): core h owns expert h's
weight [1024,1024] and processes exactly the rows routed to expert h. The host
computes the per-expert row lists from `indices`, gathers each expert's rows
into a zero-padded transposed activation block XT_h [D, C] (C = padded max
group size), and scatters the per-core outputs back into the full [B, F]
result. This does 1/8th the FLOPs of the dense all-heads reference and loads
each expert weight exactly once, on exactly one core.

On-device per core: Y[c, f] = sum_k XT[k*128:(k+1)*128, c].T @ W[k*128:.., f]
accumulated in PSUM over the 8 k-tiles, bias added during the PSUM->SBUF
eviction. X and W are pre-cast to fp16 on the host (11-bit mantissa keeps the
absmax error ~1e-3 of output scale while halving HBM traffic and enabling the
fast PE weight-load path); accumulation stays fp32 in PSUM and bias is added
in fp32. The k loop is outermost (phased over output column halves) so the
matmul pipeline starts as soon as the first k-chunk of W/X lands.
"""

from contextlib import ExitStack

import numpy as np

import concourse.bass as bass
import concourse.tile as tile
from concourse import bacc, mybir
from concourse.bass_utils import run_bass_kernel_spmd

F32 = mybir.dt.float32
F16 = mybir.dt.float16

P = 128          # SBUF partitions / matmul tile edge
NTILE = 512      # matmul moving free dim (one fp32 PSUM bank)
KG = 2           # k-tiles per DMA chunk (pipeline granularity)


def _build(nc: bass.Bass, C: int, D: int, F: int):
    xt = nc.dram_tensor("xt", (D, C), F16, kind="ExternalInput").ap()
    w = nc.dram_tensor("w", (D, F), F16, kind="ExternalInput").ap()
    b = nc.dram_tensor("b", (P, F), F32, kind="ExternalInput").ap()
    y = nc.dram_tensor("y", (C, F), F32, kind="ExternalOutput").ap()

    KT = D // P       # k-tiles (contraction)
    NT = F // NTILE   # n-tiles (output features)
    # m-tiles over this expert's rows; the last may be partial.
    msizes = []
    off = 0
    while off < C:
        msizes.append(min(P, C - off))
        off += P
    MT = len(msizes)

    # k-tiles per DMA chunk: small leading chunks so the PE pipeline starts
    # as early as possible, larger trailing ones for DMA efficiency.
    kchunks = [1, 1, 2]
    while sum(kchunks) < KT:
        kchunks.append(2)
    kchunks = kchunks[: next(i for i, s in
                             enumerate(np.cumsum(kchunks)) if s >= KT) + 1]
    kchunks[-1] = KT - sum(kchunks[:-1])

    # DRAM views with the k-tile dim split out: [KT, P, cols]
    w3 = w.rearrange("(k p) f -> k p f", p=P)
    x3 = xt.rearrange("(k p) c -> k p c", p=P)

    with tile.TileContext(nc) as tc, ExitStack() as ctx:
        wp = ctx.enter_context(tc.tile_pool(name="wp", bufs=len(kchunks)))
        xp = ctx.enter_context(tc.tile_pool(name="xp", bufs=len(kchunks)))
        bp = ctx.enter_context(tc.tile_pool(name="bp", bufs=1))
        pp = ctx.enter_context(tc.tile_pool(name="pp", bufs=8, space="PSUM"))
        yp = ctx.enter_context(tc.tile_pool(name="yp", bufs=4))

        # W chunks stream on the SP HWDGE ring, X chunks on the ACT ring
        # (rings are FIFO, so chunk c fully lands before chunk c+1 and the
        # PE can start on chunk 0 while the rest stream in). The bias load
        # rides at the tail of the ACT ring - it is only needed at the
        # first PSUM eviction.
        w_c = []
        x_c = []
        k0 = 0
        for c, kg in enumerate(kchunks):
            wt = wp.tile([P, kg * F], F16, name=f"w{c}", tag=f"w{c}")
            nc.sync.dma_start(
                wt[:], w3[k0:k0 + kg, :, :].rearrange("k p f -> p k f")
            )
            w_c.append(wt)
            xtt = xp.tile([P, kg * C], F16, name=f"x{c}", tag=f"x{c}")
            nc.scalar.dma_start(
                xtt[:], x3[k0:k0 + kg, :, :].rearrange("k p c -> p k c")
            )
            x_c.append(xtt)
            k0 += kg
        bias_t = bp.tile([P, F], F32)
        nc.scalar.dma_start(bias_t[:], b[:])

        kmap = []  # k -> (chunk, index within chunk)
        k0 = 0
        for c, kg in enumerate(kchunks):
            for ki in range(kg):
                kmap.append((c, ki))
            k0 += kg

        def mm(ps_ap, m, moff, msz, k, n):
            c, ki = kmap[k]
            nc.tensor.matmul(
                ps_ap,
                lhsT=x_c[c][:, ki * C + moff:ki * C + moff + msz],
                rhs=w_c[c][:, ki * F + n * NTILE:ki * F + (n + 1) * NTILE],
                start=(k == 0),
                stop=(k == KT - 1),
            )

        def evict(ps_ap, m, moff, msz, n):
            yt = yp.tile([P, NTILE], F32, name=f"yt{n}_{m}", tag="y")
            nc.vector.tensor_add(
                yt[:msz, :], ps_ap, bias_t[:msz, n * NTILE:(n + 1) * NTILE]
            )
            nc.scalar.dma_start(
                y[moff:moff + msz, n * NTILE:(n + 1) * NTILE], yt[:msz, :]
            )

        # Phase n=0: k outer, m inner - each W/X chunk is consumed as soon
        # as it lands while the MT PSUM banks accumulate in parallel.
        ps0 = [pp.tile([P, NTILE], F32, name=f"ps0_{m}", tag="ps")
               for m in range(MT)]
        for k in range(KT):
            moff = 0
            for m, msz in enumerate(msizes):
                mm(ps0[m][:msz, :], m, moff, msz, k, 0)
                moff += msz
        # Phase n=1: everything is resident - m outer so each m-tile's
        # eviction and output DMA overlaps the next m-tile's matmuls, and
        # the phase-0 evictions overlap phase-1 compute.
        moff = 0
        for m, msz in enumerate(msizes):
            evict(ps0[m][:msz, :], m, moff, msz, 0)
            ps1 = pp.tile([P, NTILE], F32, name=f"ps1_{m}", tag="ps")
            for k in range(KT):
                mm(ps1[:msz, :], m, moff, msz, k, 1)
            evict(ps1[:msz, :], m, moff, msz, 1)
            moff += msz


LAST_PROFILE = {}


def kernel(inputs, indices, kernel, bias, _trace=False):
    x = np.ascontiguousarray(np.asarray(inputs), dtype=np.float32)
    idx = np.asarray(indices).astype(np.int64)
    wk = np.asarray(kernel, dtype=np.float32)
    bv = np.asarray(bias, dtype=np.float32)

    B, D = x.shape
    H, _, F = wk.shape

    rows = [np.nonzero(idx == h)[0] for h in range(H)]
    maxc = max(len(r) for r in rows)
    C = max(((maxc + 15) // 16) * 16, 16)

    in_maps = []
    for h in range(H):
        r = rows[h]
        xt = np.zeros((D, C), dtype=np.float16)
        xt[:, :len(r)] = x[r].T
        in_maps.append({
            "xt": xt,
            "w": wk[h].astype(np.float16),
            "b": np.broadcast_to(bv[h], (P, F)).copy(),
        })

    nc = bacc.Bacc(
        "TRN2", target_bir_lowering=False, debug=False, num_devices=H
    )
    _build(nc, C, D, F)
    nc.compile()

    trace_kwargs = (
        {"trace": True, "trace_cores": list(range(H)), "stitch_traces": False}
        if _trace
        else {}
    )
    res = run_bass_kernel_spmd(nc, in_maps, core_ids=list(range(H)), **trace_kwargs)
    if _trace:
        LAST_PROFILE.clear()
        LAST_PROFILE.update(
            exec_time_ns=res.exec_time_ns,
            mean_exec_time_ns=res.mean_exec_time_ns,
            max_exec_time_core_id=res.max_exec_time_core_id,
            trace=res.instructions_and_trace[1] if res.instructions_and_trace else None,
            profile_json=res.profile_json,
        )

    out = np.empty((B, F), dtype=np.float32)
    for h in range(H):
        r = rows[h]
        out[r] = res.results[h]["y"][:len(r)]
    return out


# revision 14
# speedup vs baseline: 1.2879x; 1.0067x over previous
"""MoE routed dense layer (nn_MultiHeadDense): y[b] = x[b] @ W[idx[b]] + bias[idx[b]].

Full shapes: inputs [4096,1024] f32, indices [4096] int, kernel [8,1024,1024] f32,
bias [8,1024] f32 -> out [4096,1024] f32.

Sharding strategy (expert-parallel, H == n_cores == 8): core h owns expert h's
weight [1024,1024] and processes exactly the rows routed to expert h. The host
computes the per-expert row lists from `indices`, gathers each expert's rows
into a zero-padded transposed activation block XT_h [D, C] (C = padded max
group size), and scatters the per-core outputs back into the full [B, F]
result. This does 1/8th the FLOPs of the dense all-heads reference and loads
each expert weight exactly once, on exactly one core.

On-device per core: Y[c, f] = sum_k XT[k*128:(k+1)*128, c].T @ W[k*128:.., f]
accumulated in PSUM over the 8 k-tiles, bias added during the PSUM->SBUF
eviction. X and W are pre-cast to fp16 on the host (11-bit mantissa keeps the
absmax error ~1e-3 of output scale while halving HBM traffic and enabling the
fast PE weight-load path); accumulation stays fp32 in PSUM and bias is added
in fp32. The k loop is outermost (phased over output column halves) so the
matmul pipeline starts as soon as the first k-chunk of W/X lands.
"""

from contextlib import ExitStack

import numpy as np

import concourse.bass as bass
import concourse.tile as tile
from concourse import bacc, mybir
from concourse.bass_utils import run_bass_kernel_spmd

F32 = mybir.dt.float32
F16 = mybir.dt.float16

P = 128          # SBUF partitions / matmul tile edge
NTILE = 512      # matmul moving free dim (one fp32 PSUM bank)
KG = 2           # k-tiles per DMA chunk (pipeline granularity)


def _build(nc: bass.Bass, C: int, D: int, F: int):
    xt = nc.dram_tensor("xt", (D, C), F16, kind="ExternalInput").ap()
    w = nc.dram_tensor("w", (D, F), F16, kind="ExternalInput").ap()
    b = nc.dram_tensor("b", (P, F), F32, kind="ExternalInput").ap()
    y = nc.dram_tensor("y", (C, F), F32, kind="ExternalOutput").ap()

    KT = D // P       # k-tiles (contraction)
    NT = F // NTILE   # n-tiles (output features)
    # m-tiles over this expert's rows; the last may be partial.
    msizes = []
    off = 0
    while off < C:
        msizes.append(min(P, C - off))
        off += P
    MT = len(msizes)

    # k-tiles per DMA chunk: small leading chunks so the PE pipeline starts
    # as early as possible, larger trailing ones for DMA efficiency.
    kchunks = [1, 1, 2]
    while sum(kchunks) < KT:
        kchunks.append(2)
    kchunks = kchunks[: next(i for i, s in
                             enumerate(np.cumsum(kchunks)) if s >= KT) + 1]
    kchunks[-1] = KT - sum(kchunks[:-1])

    # DRAM views with the k-tile dim split out: [KT, P, cols]
    w3 = w.rearrange("(k p) f -> k p f", p=P)
    x3 = xt.rearrange("(k p) c -> k p c", p=P)

    moffs = []
    off = 0
    for msz in msizes:
        moffs.append(off)
        off += msz

    with tile.TileContext(nc) as tc, ExitStack() as ctx:
        wp = ctx.enter_context(tc.tile_pool(name="wp", bufs=1))
        xp = ctx.enter_context(tc.tile_pool(name="xp", bufs=1))
        bp = ctx.enter_context(tc.tile_pool(name="bp", bufs=1))
        zp = ctx.enter_context(tc.tile_pool(name="zp", bufs=1))
        pp = ctx.enter_context(tc.tile_pool(name="pp", bufs=8, space="PSUM"))
        yp = ctx.enter_context(tc.tile_pool(name="yp", bufs=4))

        # W chunks stream on the SP HWDGE ring in n-half order (the n=0
        # halves of every k chunk first): phase 0 only becomes gated on half
        # the weight bytes, and the n=1 halves stream in during phase-0
        # compute. X chunks ride the ACT ring, bias at its tail (only needed
        # at the first PSUM eviction). Rings are FIFO, so chunk c fully
        # lands before chunk c+1 and the PE starts on chunk 0 early.
        w_c = {}  # (n, chunk) -> tile
        x_c = []
        for n in range(NT):
            k0 = 0
            for c, kg in enumerate(kchunks):
                wt = wp.tile([P, kg * NTILE], F16, name=f"w{n}_{c}",
                             tag=f"w{n}_{c}")
                nc.sync.dma_start(
                    wt[:],
                    w3[k0:k0 + kg, :, n * NTILE:(n + 1) * NTILE]
                    .rearrange("k p f -> p k f"),
                )
                w_c[(n, c)] = wt
                k0 += kg
        k0 = 0
        for c, kg in enumerate(kchunks):
            xtt = xp.tile([P, kg * C], F16, name=f"x{c}", tag=f"x{c}")
            nc.scalar.dma_start(
                xtt[:], x3[k0:k0 + kg, :, :].rearrange("k p c -> p k c")
            )
            x_c.append(xtt)
            k0 += kg
        bias_t = bp.tile([P, F], F32)
        nc.scalar.dma_start(bias_t[:], b[:])

        kmap = []  # k -> (chunk, index within chunk)
        for c, kg in enumerate(kchunks):
            kmap.extend((c, ki) for ki in range(kg))

        # PE warmup: ~3.5us of zero matmuls into a scratch PSUM bank while
        # the first chunks stream in, so the HAM clock gate is already at
        # full rate (2.4 GHz) when the real matmuls start.
        zt = zp.tile([P, NTILE], F16)
        nc.gpsimd.memset(zt[:], 0.0)
        ps_warm = pp.tile([P, NTILE], F32, name="ps_warm", tag="ps")
        for _ in range(8):
            nc.tensor.matmul(ps_warm[:], lhsT=zt[:, :P], rhs=zt[:],
                             start=True, stop=True)

        def mm(ps_ap, msz, moff, k, n):
            c, ki = kmap[k]
            nc.tensor.matmul(
                ps_ap,
                lhsT=x_c[c][:, ki * C + moff:ki * C + moff + msz],
                rhs=w_c[(n, c)][:, ki * NTILE:(ki + 1) * NTILE],
                start=(k == 0),
                stop=(k == KT - 1),
            )

        def evict(ps_ap, m, msz, moff, n):
            yt = yp.tile([P, NTILE], F32, name=f"yt{n}_{m}", tag="y")
            nc.vector.tensor_add(
                yt[:msz, :], ps_ap, bias_t[:msz, n * NTILE:(n + 1) * NTILE]
            )
            nc.scalar.dma_start(
                y[moff:moff + msz, n * NTILE:(n + 1) * NTILE], yt[:msz, :]
            )

        # Both phases are k-outer / m-inner so the PE consumes each chunk as
        # it lands and the per-m PSUM banks accumulate in parallel. For the
        # last chunk the loop is m-outer so each m-tile finishes (stop=True)
        # a few matmuls before the next, letting the DVE evictions and
        # output DMAs overlap the remaining matmuls instead of bunching
        # after the last one.
        for n in range(NT):
            ps = [pp.tile([P, NTILE], F32, name=f"ps{n}_{m}", tag="ps")
                  for m in range(MT)]
            klast = KT - kchunks[-1]
            for k in range(klast):
                for m, msz in enumerate(msizes):
                    mm(ps[m][:msz, :], msz, moffs[m], k, n)
            for m, msz in enumerate(msizes):
                for k in range(klast, KT):
                    mm(ps[m][:msz, :], msz, moffs[m], k, n)
                evict(ps[m][:msz, :], m, msz, moffs[m], n)


LAST_PROFILE = {}


def kernel(inputs, indices, kernel, bias, _trace=False):
    x = np.ascontiguousarray(np.asarray(inputs), dtype=np.float32)
    idx = np.asarray(indices).astype(np.int64)
    wk = np.asarray(kernel, dtype=np.float32)
    bv = np.asarray(bias, dtype=np.float32)

    B, D = x.shape
    H, _, F = wk.shape

    rows = [np.nonzero(idx == h)[0] for h in range(H)]
    maxc = max(len(r) for r in rows)
    C = max(((maxc + 15) // 16) * 16, 16)

    in_maps = []
    for h in range(H):
        r = rows[h]
        xt = np.zeros((D, C), dtype=np.float16)
        xt[:, :len(r)] = x[r].T
        in_maps.append({
            "xt": xt,
            "w": wk[h].astype(np.float16),
            "b": np.broadcast_to(bv[h], (P, F)).copy(),
        })

    nc = bacc.Bacc(
        "TRN2", target_bir_lowering=False, debug=False, num_devices=H,
        enable_asserts=False,
    )
    _build(nc, C, D, F)
    nc.compile()

    trace_kwargs = (
        {"trace": True, "trace_cores": list(range(H)), "stitch_traces": False}
        if _trace
        else {}
    )
    res = run_bass_kernel_spmd(nc, in_maps, core_ids=list(range(H)), **trace_kwargs)
    if _trace:
        LAST_PROFILE.clear()
        LAST_PROFILE.update(
            exec_time_ns=res.exec_time_ns,
            mean_exec_time_ns=res.mean_exec_time_ns,
            max_exec_time_core_id=res.max_exec_time_core_id,
            trace=res.instructions_and_trace[1] if res.instructions_and_trace else None,
            profile_json=res.profile_json,
        )

    out = np.empty((B, F), dtype=np.float32)
    for h in range(H):
        r = rows[h]
        out[r] = res.results[h]["y"][:len(r)]
    return out


# revision 16
# speedup vs baseline: 1.3838x; 1.0744x over previous
"""MoE routed dense layer (nn_MultiHeadDense): y[b] = x[b] @ W[idx[b]] + bias[idx[b]].

Full shapes: inputs [4096,1024] f32, indices [4096] int, kernel [8,1024,1024] f32,
bias [8,1024] f32 -> out [4096,1024] f32.

Sharding strategy (expert-parallel, H == n_cores == 8): core h owns expert h's
weight [1024,1024] and processes exactly the rows routed to expert h. The host
computes the per-expert row lists from `indices`, gathers each expert's rows
into a zero-padded transposed activation block XT_h [D, C] (C = padded max
group size), and scatters the per-core outputs back into the full [B, F]
result. This does 1/8th the FLOPs of the dense all-heads reference and loads
each expert weight exactly once, on exactly one core.

On-device per core: Y[c, f] = sum_k XT[k*128:(k+1)*128, c].T @ W[k*128:.., f]
accumulated in PSUM over the 8 k-tiles, bias added during the PSUM->SBUF
eviction. X and W are pre-cast to fp16 on the host (11-bit mantissa keeps the
absmax error ~1e-3 of output scale while halving HBM traffic and enabling the
fast PE weight-load path); accumulation stays fp32 in PSUM and bias is added
in fp32. The k loop is outermost (phased over output column halves) so the
matmul pipeline starts as soon as the first k-chunk of W/X lands.
"""

from contextlib import ExitStack

import numpy as np

import concourse.bass as bass
import concourse.tile as tile
from concourse import bacc, mybir
from concourse.bass_utils import run_bass_kernel_spmd

F32 = mybir.dt.float32
F16 = mybir.dt.float16

P = 128          # SBUF partitions / matmul tile edge
NTILE = 512      # matmul moving free dim (one fp32 PSUM bank)
KG = 2           # k-tiles per DMA chunk (pipeline granularity)


def _plan(C, D, F):
    """Shared host/device plan: k chunks, m tiles, packed buffer offsets."""
    KT = D // P
    NT = F // NTILE
    kchunks = [1, 1]
    while sum(kchunks) < KT:
        kchunks.append(min(2, KT - sum(kchunks)))
    msizes = []
    off = 0
    while off < C:
        msizes.append(min(P, C - off))
        off += P
    moffs = list(np.cumsum([0] + msizes[:-1]))
    return KT, NT, kchunks, msizes, moffs


def _build(nc: bass.Bass, C: int, D: int, F: int):
    KT, NT, kchunks, msizes, moffs = _plan(C, D, F)
    KC = len(kchunks)

    # w / xt are host-packed streams: consecutive [P, kg*cols] blocks laid
    # out contiguously in exactly DMA order, so every transfer is a single
    # fully-contiguous block with multi-KB per-partition lines.
    wf = nc.dram_tensor("w", (KT * P * F,), F16, kind="ExternalInput").ap()
    xf = nc.dram_tensor("xt", (KT * P * C,), F16, kind="ExternalInput").ap()
    b = nc.dram_tensor("b", (P, F), F32, kind="ExternalInput").ap()
    y = nc.dram_tensor("y", (C, F), F32, kind="ExternalOutput").ap()

    with tile.TileContext(nc) as tc, ExitStack() as ctx:
        wp = ctx.enter_context(tc.tile_pool(name="wp", bufs=1))
        xp = ctx.enter_context(tc.tile_pool(name="xp", bufs=1))
        bp = ctx.enter_context(tc.tile_pool(name="bp", bufs=1))
        zp = ctx.enter_context(tc.tile_pool(name="zp", bufs=1))
        pp = ctx.enter_context(tc.tile_pool(name="pp", bufs=8, space="PSUM"))
        yp = ctx.enter_context(tc.tile_pool(name="yp", bufs=4))

        # W chunks stream on the SP HWDGE ring in n-half order (the n=0
        # halves of every k chunk first): phase 0 only becomes gated on half
        # the weight bytes, and the n=1 halves stream in during phase-0
        # compute. X chunks ride the ACT ring, bias at its tail (only needed
        # at the first PSUM eviction). Rings are FIFO, so chunk c fully
        # lands before chunk c+1 and the PE starts on chunk 0 early.
        w_c = {}  # (n, chunk) -> tile
        x_c = []
        woff = 0
        for n in range(NT):
            for c, kg in enumerate(kchunks):
                wt = wp.tile([P, kg * NTILE], F16, name=f"w{n}_{c}",
                             tag=f"w{n}_{c}")
                nc.sync.dma_start(
                    wt[:],
                    wf[woff:woff + P * kg * NTILE]
                    .rearrange("(p q) -> p q", p=P),
                )
                w_c[(n, c)] = wt
                woff += P * kg * NTILE
        xoff = 0
        for c, kg in enumerate(kchunks):
            xtt = xp.tile([P, kg * C], F16, name=f"x{c}", tag=f"x{c}")
            nc.scalar.dma_start(
                xtt[:],
                xf[xoff:xoff + P * kg * C].rearrange("(p q) -> p q", p=P),
            )
            x_c.append(xtt)
            xoff += P * kg * C
        bias_t = bp.tile([P, F], F32)
        nc.scalar.dma_start(bias_t[:], b[:])

        kmap = []  # k -> (chunk, index within chunk)
        for c, kg in enumerate(kchunks):
            kmap.extend((c, ki) for ki in range(kg))

        # PE warmup: ~3.5us of zero matmuls into a scratch PSUM bank while
        # the first chunks stream in, so the HAM clock gate is already at
        # full rate (2.4 GHz) when the real matmuls start.
        zt = zp.tile([P, NTILE], F16)
        nc.gpsimd.memset(zt[:], 0.0)
        ps_warm = pp.tile([P, NTILE], F32, name="ps_warm", tag="ps")
        for _ in range(8):
            nc.tensor.matmul(ps_warm[:], lhsT=zt[:, :P], rhs=zt[:],
                             start=True, stop=True)

        def mm(ps_ap, msz, moff, k, n):
            c, ki = kmap[k]
            nc.tensor.matmul(
                ps_ap,
                lhsT=x_c[c][:, ki * C + moff:ki * C + moff + msz],
                rhs=w_c[(n, c)][:, ki * NTILE:(ki + 1) * NTILE],
                start=(k == 0),
                stop=(k == KT - 1),
            )

        def evict(ps_ap, m, msz, moff, n):
            yt = yp.tile([P, NTILE], F32, name=f"yt{n}_{m}", tag="y")
            nc.vector.tensor_add(
                yt[:msz, :], ps_ap, bias_t[:msz, n * NTILE:(n + 1) * NTILE]
            )
            nc.scalar.dma_start(
                y[moff:moff + msz, n * NTILE:(n + 1) * NTILE], yt[:msz, :]
            )

        # Full (128-row) m-tiles run k-outer / m-inner so the PE consumes
        # each chunk as it lands and per-m PSUM banks accumulate in
        # parallel; the last chunk goes m-outer so evictions start a couple
        # of matmuls apart. The partial m-tile (if any) runs as its own
        # k-block at the end of the phase: its different tile_size doesn't
        # perturb the main matmul stream, and its matmuls overlap the full
        # tiles' DVE evictions + output DMAs, leaving only its own tiny
        # eviction as the phase tail.
        MF = sum(1 for s in msizes if s == P)
        for n in range(NT):
            ps = [pp.tile([P, NTILE], F32, name=f"ps{n}_{m}", tag="ps")
                  for m in range(len(msizes))]
            klast = KT - kchunks[-1]
            for k in range(klast):
                for m in range(MF):
                    mm(ps[m][:P, :], P, moffs[m], k, n)
            for m in range(MF):
                for k in range(klast, KT):
                    mm(ps[m][:P, :], P, moffs[m], k, n)
                evict(ps[m][:P, :], m, P, moffs[m], n)
            for m in range(MF, len(msizes)):
                msz = msizes[m]
                for k in range(KT):
                    mm(ps[m][:msz, :], msz, moffs[m], k, n)
                evict(ps[m][:msz, :], m, msz, moffs[m], n)


LAST_PROFILE = {}


def kernel(inputs, indices, kernel, bias, _trace=False):
    x = np.ascontiguousarray(np.asarray(inputs), dtype=np.float32)
    idx = np.asarray(indices).astype(np.int64)
    wk = np.asarray(kernel, dtype=np.float32)
    bv = np.asarray(bias, dtype=np.float32)

    B, D = x.shape
    H, _, F = wk.shape

    rows = [np.nonzero(idx == h)[0] for h in range(H)]
    maxc = max(len(r) for r in rows)
    C = max(((maxc + 15) // 16) * 16, 16)

    KT, NT, kchunks, _, _ = _plan(C, D, F)

    def pack_w(w16):
        # blocks in stream order: for n-half, for k-chunk: [P, kg*NTILE]
        # where block[p, ki*NTILE + f] = W[(k0+ki)*P + p, n*NTILE + f]
        parts = []
        for n in range(NT):
            k0 = 0
            for kg in kchunks:
                blk = w16[k0 * P:(k0 + kg) * P, n * NTILE:(n + 1) * NTILE]
                parts.append(
                    blk.reshape(kg, P, NTILE).transpose(1, 0, 2).reshape(-1)
                )
                k0 += kg
        return np.concatenate(parts)

    def pack_x(xt16):
        # blocks: for k-chunk: [P, kg*C], block[p, ki*C + c] = XT[(k0+ki)*P+p, c]
        parts = []
        k0 = 0
        for kg in kchunks:
            blk = xt16[k0 * P:(k0 + kg) * P, :]
            parts.append(blk.reshape(kg, P, C).transpose(1, 0, 2).reshape(-1))
            k0 += kg
        return np.concatenate(parts)

    in_maps = []
    for h in range(H):
        r = rows[h]
        xt = np.zeros((D, C), dtype=np.float16)
        xt[:, :len(r)] = x[r].T
        in_maps.append({
            "xt": pack_x(xt),
            "w": pack_w(wk[h].astype(np.float16)),
            "b": np.broadcast_to(bv[h], (P, F)).copy(),
        })

    nc = bacc.Bacc(
        "TRN2", target_bir_lowering=False, debug=False, num_devices=H,
        enable_asserts=False,
    )
    _build(nc, C, D, F)
    nc.compile()

    trace_kwargs = (
        {"trace": True, "trace_cores": list(range(H)), "stitch_traces": False}
        if _trace
        else {}
    )
    res = run_bass_kernel_spmd(nc, in_maps, core_ids=list(range(H)), **trace_kwargs)
    if _trace:
        LAST_PROFILE.clear()
        LAST_PROFILE.update(
            exec_time_ns=res.exec_time_ns,
            mean_exec_time_ns=res.mean_exec_time_ns,
            max_exec_time_core_id=res.max_exec_time_core_id,
            trace=res.instructions_and_trace[1] if res.instructions_and_trace else None,
            profile_json=res.profile_json,
        )

    out = np.empty((B, F), dtype=np.float32)
    for h in range(H):
        r = rows[h]
        out[r] = res.results[h]["y"][:len(r)]
    return out


# revision 18
# speedup vs baseline: 1.4688x; 1.0615x over previous
"""MoE routed dense layer (nn_MultiHeadDense): y[b] = x[b] @ W[idx[b]] + bias[idx[b]].

Full shapes: inputs [4096,1024] f32, indices [4096] int, kernel [8,1024,1024] f32,
bias [8,1024] f32 -> out [4096,1024] f32.

Sharding strategy (expert-parallel, H == n_cores == 8): core h owns expert h's
weight [1024,1024] and processes exactly the rows routed to expert h. The host
computes the per-expert row lists from `indices`, gathers each expert's rows
into a zero-padded transposed activation block XT_h [D, C] (C = padded max
group size), and scatters the per-core outputs back into the full [B, F]
result. This does 1/8th the FLOPs of the dense all-heads reference and loads
each expert weight exactly once, on exactly one core.

On-device per core: Y[c, f] = sum_k XT[k*128:(k+1)*128, c].T @ W[k*128:.., f]
accumulated in PSUM over the 8 k-tiles, bias added during the PSUM->SBUF
eviction. X and W are pre-cast to fp16 on the host (11-bit mantissa keeps the
absmax error ~1e-3 of output scale while halving HBM traffic and enabling the
fast PE weight-load path); accumulation stays fp32 in PSUM and bias is added
in fp32. The k loop is outermost (phased over output column halves) so the
matmul pipeline starts as soon as the first k-chunk of W/X lands.
"""

from contextlib import ExitStack

import numpy as np

import concourse.bass as bass
import concourse.tile as tile
from concourse import bacc, mybir
from concourse.bass_utils import run_bass_kernel_spmd

F32 = mybir.dt.float32
F16 = mybir.dt.float16

P = 128          # SBUF partitions / matmul tile edge
NTILE = 512      # matmul moving free dim (one fp32 PSUM bank)
KG = 2           # k-tiles per DMA chunk (pipeline granularity)


def _plan(C, D, F):
    """Shared host/device plan: k chunks, m tiles, packed buffer offsets."""
    KT = D // P
    NT = F // NTILE
    kchunks = [1, 1]
    while sum(kchunks) < KT:
        kchunks.append(min(2, KT - sum(kchunks)))
    msizes = []
    off = 0
    while off < C:
        msizes.append(min(P, C - off))
        off += P
    moffs = list(np.cumsum([0] + msizes[:-1]))
    return KT, NT, kchunks, msizes, moffs


def _build(nc: bass.Bass, C: int, D: int, F: int):
    KT, NT, kchunks, msizes, moffs = _plan(C, D, F)
    KC = len(kchunks)

    # w / xt are host-packed streams: consecutive [P, kg*cols] blocks laid
    # out contiguously in exactly DMA order, so every transfer is a single
    # fully-contiguous block with multi-KB per-partition lines.
    wf = nc.dram_tensor("w", (KT * P * F,), F16, kind="ExternalInput").ap()
    xf = nc.dram_tensor("xt", (KT * P * C,), F16, kind="ExternalInput").ap()
    b = nc.dram_tensor("b", (P, F), F32, kind="ExternalInput").ap()
    y = nc.dram_tensor("y", (C, F), F32, kind="ExternalOutput").ap()

    with tile.TileContext(nc) as tc, ExitStack() as ctx:
        wp = ctx.enter_context(tc.tile_pool(name="wp", bufs=1))
        xp = ctx.enter_context(tc.tile_pool(name="xp", bufs=1))
        bp = ctx.enter_context(tc.tile_pool(name="bp", bufs=1))
        zp = ctx.enter_context(tc.tile_pool(name="zp", bufs=1))
        pp = ctx.enter_context(tc.tile_pool(name="pp", bufs=8, space="PSUM"))
        yp = ctx.enter_context(tc.tile_pool(name="yp", bufs=4))

        # W chunks stream whole-F on the SP HWDGE ring; X chunks on the ACT
        # ring, bias at its tail (only needed at the first PSUM eviction).
        # Rings are FIFO, so chunk c fully lands before chunk c+1 and the
        # PE starts on chunk 0 early. Steady state consumes one k-tile as 8
        # N=512 matmuls (~1.7us) vs ~1.2us of DMA per k-tile, so after the
        # fill the pipeline is PE-bound with no matmul stalls.
        w_c = []
        x_c = []
        woff = 0
        xoff = 0
        for c, kg in enumerate(kchunks):
            wt = wp.tile([P, kg * F], F16, name=f"w{c}", tag=f"w{c}")
            nc.sync.dma_start(
                wt[:], wf[woff:woff + P * kg * F].rearrange("(p q) -> p q", p=P)
            )
            w_c.append(wt)
            woff += P * kg * F
            xtt = xp.tile([P, kg * C], F16, name=f"x{c}", tag=f"x{c}")
            nc.scalar.dma_start(
                xtt[:],
                xf[xoff:xoff + P * kg * C].rearrange("(p q) -> p q", p=P),
            )
            x_c.append(xtt)
            xoff += P * kg * C
        bias_t = bp.tile([P, F], F32)
        nc.scalar.dma_start(bias_t[:], b[:])

        kmap = []  # k -> (chunk, index within chunk)
        for c, kg in enumerate(kchunks):
            kmap.extend((c, ki) for ki in range(kg))

        # PE warmup: ~3.5us of zero matmuls into a scratch PSUM bank while
        # the first chunks stream in, so the HAM clock gate is already at
        # full rate (2.4 GHz) when the real matmuls start.
        zt = zp.tile([P, NTILE], F16)
        nc.gpsimd.memset(zt[:], 0.0)
        ps_warm = pp.tile([P, NTILE], F32, name="ps_warm", tag="ps")
        for _ in range(8):
            nc.tensor.matmul(ps_warm[:], lhsT=zt[:, :P], rhs=zt[:],
                             start=True, stop=True)

        def mm(ps_ap, msz, moff, k, n):
            c, ki = kmap[k]
            nc.tensor.matmul(
                ps_ap,
                lhsT=x_c[c][:, ki * C + moff:ki * C + moff + msz],
                rhs=w_c[c][:, ki * F + n * NTILE:ki * F + (n + 1) * NTILE],
                start=(k == 0),
                stop=(k == KT - 1),
            )

        def evict(ps_ap, m, msz, moff, n):
            yt = yp.tile([P, NTILE], F32, name=f"yt{n}_{m}", tag="y")
            nc.vector.tensor_add(
                yt[:msz, :], ps_ap, bias_t[:msz, n * NTILE:(n + 1) * NTILE]
            )
            nc.scalar.dma_start(
                y[moff:moff + msz, n * NTILE:(n + 1) * NTILE], yt[:msz, :]
            )

        # Single main pass: the 4 full m-tiles x 2 n-halves use all 8 PSUM
        # banks with k outermost. The last chunk runs m-outer so each
        # m-tile's two evictions start a few matmuls before the next
        # m-tile finishes. The partial m-tile (if any) runs as its own
        # k-block at the end: its different tile_size doesn't perturb the
        # main matmul stream, and its 2*KT matmuls overlap the full tiles'
        # DVE evictions + output DMAs, leaving only its own tiny eviction
        # as the kernel tail.
        MF = sum(1 for s in msizes if s == P)
        ps = {}
        for m in range(MF):
            for n in range(NT):
                ps[(m, n)] = pp.tile([P, NTILE], F32, name=f"ps{m}_{n}",
                                     tag="ps")
        klast = KT - kchunks[-1]
        for k in range(klast):
            for m in range(MF):
                for n in range(NT):
                    mm(ps[(m, n)][:P, :], P, moffs[m], k, n)
        for m in range(MF):
            for k in range(klast, KT):
                for n in range(NT):
                    mm(ps[(m, n)][:P, :], P, moffs[m], k, n)
            for n in range(NT):
                evict(ps[(m, n)][:P, :], m, P, moffs[m], n)
        for m in range(MF, len(msizes)):
            msz = msizes[m]
            for n in range(NT):
                psr = pp.tile([P, NTILE], F32, name=f"psr{m}_{n}", tag="ps")
                for k in range(KT):
                    mm(psr[:msz, :], msz, moffs[m], k, n)
                evict(psr[:msz, :], m, msz, moffs[m], n)


LAST_PROFILE = {}


def kernel(inputs, indices, kernel, bias, _trace=False):
    x = np.ascontiguousarray(np.asarray(inputs), dtype=np.float32)
    idx = np.asarray(indices).astype(np.int64)
    wk = np.asarray(kernel, dtype=np.float32)
    bv = np.asarray(bias, dtype=np.float32)

    B, D = x.shape
    H, _, F = wk.shape

    rows = [np.nonzero(idx == h)[0] for h in range(H)]
    maxc = max(len(r) for r in rows)
    C = max(((maxc + 15) // 16) * 16, 16)

    KT, NT, kchunks, _, _ = _plan(C, D, F)

    def pack_w(w16):
        # blocks in stream order: per k-chunk [P, kg*F] where
        # block[p, ki*F + f] = W[(k0+ki)*P + p, f]
        parts = []
        k0 = 0
        for kg in kchunks:
            blk = w16[k0 * P:(k0 + kg) * P, :]
            parts.append(blk.reshape(kg, P, F).transpose(1, 0, 2).reshape(-1))
            k0 += kg
        return np.concatenate(parts)

    def pack_x(xt16):
        # blocks: for k-chunk: [P, kg*C], block[p, ki*C + c] = XT[(k0+ki)*P+p, c]
        parts = []
        k0 = 0
        for kg in kchunks:
            blk = xt16[k0 * P:(k0 + kg) * P, :]
            parts.append(blk.reshape(kg, P, C).transpose(1, 0, 2).reshape(-1))
            k0 += kg
        return np.concatenate(parts)

    in_maps = []
    for h in range(H):
        r = rows[h]
        xt = np.zeros((D, C), dtype=np.float16)
        xt[:, :len(r)] = x[r].T
        in_maps.append({
            "xt": pack_x(xt),
            "w": pack_w(wk[h].astype(np.float16)),
            "b": np.broadcast_to(bv[h], (P, F)).copy(),
        })

    nc = bacc.Bacc(
        "TRN2", target_bir_lowering=False, debug=False, num_devices=H,
        enable_asserts=False,
    )
    _build(nc, C, D, F)
    nc.compile()

    trace_kwargs = (
        {"trace": True, "trace_cores": list(range(H)), "stitch_traces": False}
        if _trace
        else {}
    )
    res = run_bass_kernel_spmd(nc, in_maps, core_ids=list(range(H)), **trace_kwargs)
    if _trace:
        LAST_PROFILE.clear()
        LAST_PROFILE.update(
            exec_time_ns=res.exec_time_ns,
            mean_exec_time_ns=res.mean_exec_time_ns,
            max_exec_time_core_id=res.max_exec_time_core_id,
            trace=res.instructions_and_trace[1] if res.instructions_and_trace else None,
            profile_json=res.profile_json,
        )

    out = np.empty((B, F), dtype=np.float32)
    for h in range(H):
        r = rows[h]
        out[r] = res.results[h]["y"][:len(r)]
    return out
